# revision 19
# baseline (speedup 1.0000x reference)
"""Trainium2 Bass kernel for nn_CcLoss (gnn_message_passing).

Full inputs: features [64, 1024, 128] f32, tau scalar f32.
Data-parallel over batch B across 8 NeuronCores (8 samples per core).

Per sample b (on device):
  fn    = f / ||f||_rows                   (bf16, Act with per-row scale)
  fnT   = DMA XBAR transpose of fn (8 [128,128] block transposes)
  sim   = fnT.T @ fnT                      (PE bf16 -> fp32 PSUM)
  mask  = (sim > tau) in {0,1} written as fp8e4 by DVE+Pool is_gt,
          with fused row-degree accumulation (accum_out).
  protoT = sum_j (f8_pair_j).T @ mask_pair_j   via fp8 DoubleRow matmuls
          (K=256 per instruction, f8 stationary -> 16 weight loads/sample).
          f8 = Q8(f) + r8(f) hi/lo fp8 split keeps bf16-level precision.
          Output is proto TRANSPOSED: [d, q] in PSUM fp32.
  protoS = protoT * rdeg_rep (bf16) with fused gtsum[d] accumulation.
  stats : ff (norm pass), pf = sum protoS*fT, pp = sum protoS^2, per-d cols.
Host combines stats into MSE + Pearson loss (exact algebra of the reference).
"""

import numpy as np

B, P, D = 64, 1024, 128
NCORES = 8
BLOC = B // NCORES          # samples per core
NT = P // 128               # 128-row tiles per sample
SW = 11                     # stat cols per sample: 0:8 ff_t, 8 pf, 9 pp, 10 gtsum

# mask tile (kc) engine split: tiles 0..NA-1 on Act (Sign-form, S in {-1,1},
# stationary f8 pre-halved, corrected via c_col), tiles NA.. on DVE ({0,1}).
NA = 5
# sim row-tile processing order interleaves Act/DVE thresholds for pipelining
MT_ORDER = (0, 5, 1, 6, 2, 7, 3, 4)

_PROG = None


def _build_program():
    import concourse.tile as tile
    from concourse import bacc, mybir, masks

    f32 = mybir.dt.float32
    bf16 = mybir.dt.bfloat16
    fp8 = mybir.dt.float8e4
    AF = mybir.ActivationFunctionType
    OP = mybir.AluOpType
    DR = mybir.MatmulPerfMode.DoubleRow

    nc = bacc.Bacc(
        "TRN2",
        target_bir_lowering=False,
        debug=False,
        enable_asserts=False,
        num_devices=NCORES,
    )
    feats = nc.dram_tensor("features", [BLOC, P, D], f32, kind="ExternalInput").ap()
    tau_d = nc.dram_tensor("tau", [1, 1], f32, kind="ExternalInput").ap()
    out_d = nc.dram_tensor("out", [128, BLOC * SW], f32, kind="ExternalOutput").ap()
    rrow_d = nc.dram_tensor("rrow_scratch", [BLOC, P], bf16, kind="Internal").ap()

    with tile.TileContext(nc) as tc:
        from contextlib import ExitStack

        with ExitStack() as ctx:
            const = ctx.enter_context(tc.tile_pool(name="const", bufs=1))
            fpool = ctx.enter_context(tc.tile_pool(name="f", bufs=4))
            fnpool = ctx.enter_context(tc.tile_pool(name="fn", bufs=3))
            ftpool = ctx.enter_context(tc.tile_pool(name="fnT", bufs=3))
            q8pool = ctx.enter_context(tc.tile_pool(name="q8", bufs=4))
            r8pool = ctx.enter_context(tc.tile_pool(name="r8", bufs=4))
            mpool = ctx.enter_context(tc.tile_pool(name="mask", bufs=3))
            fTpool = ctx.enter_context(tc.tile_pool(name="fT", bufs=2))
            pspool = ctx.enter_context(tc.tile_pool(name="protoS", bufs=2))
            stpool = ctx.enter_context(tc.tile_pool(name="stat", bufs=4))
            smpool = ctx.enter_context(tc.tile_pool(name="small", bufs=6))
            rowpool = ctx.enter_context(tc.tile_pool(name="rows", bufs=3))
            reppool = ctx.enter_context(tc.tile_pool(name="reps", bufs=3))
            dscr = ctx.enter_context(tc.tile_pool(name="dscr", bufs=2))
            gscr = ctx.enter_context(tc.tile_pool(name="gscr", bufs=3))
            pss_pool = ctx.enter_context(tc.tile_pool(name="pss", bufs=2, space="PSUM"))
            pmm_pool = ctx.enter_context(tc.tile_pool(name="pmm", bufs=2, space="PSUM"))
            pT_pool = ctx.enter_context(tc.tile_pool(name="pT", bufs=1, space="PSUM"))

            ident16 = const.tile([128, 128], bf16)
            masks.make_identity(nc, ident16[:])
            tau_bc = const.tile([128, 1], f32)
            nc.sync.dma_start(tau_bc[:], tau_d[0, :].partition_broadcast(128))
            ntau = const.tile([128, 1], f32)
            nc.gpsimd.tensor_scalar_mul(ntau[:], tau_bc[:], -1.0)
            statall = const.tile([128, BLOC * SW], f32)

            st = {}

            def row_replicate(src8, s, tag):
                """[128,8] f32 per-tile col -> bf16 row replicated [128, P]
                via PE transpose + DRAM round-trip broadcast."""
                s16 = smpool.tile([128, 8], bf16, tag=f"{tag}16")
                nc.scalar.copy(s16[:], src8[:])
                prow = pmm_pool.tile([8, 128], bf16, tag="rowT")
                nc.tensor.matmul(prow[:], s16[:], ident16[:], is_transpose=True)
                row8 = rowpool.tile([8, 128], bf16, tag=f"{tag}r8")
                nc.scalar.copy(row8[:], prow[:])
                nc.sync.dma_start(
                    rrow_d[s].rearrange("(t p) -> t p", t=NT), row8[:]
                )
                rep = reppool.tile([128, P], bf16, tag=f"{tag}rep")
                nc.sync.dma_start(rep[:], rrow_d[s].partition_broadcast(128))
                return rep

            def stage_load(s):
                fb = fpool.tile([128, NT * 128], f32, tag="fb")
                nc.sync.dma_start(
                    fb[:].rearrange("p (t d) -> p t d", t=NT),
                    feats[s].rearrange("(t p) d -> p t d", p=128),
                )
                st[s] = {"fb": fb}

            def stage_prep(s):
                v = st[s]
                fb = v["fb"]
                statv = stpool.tile([128, SW], f32, tag="statv")
                sq = dscr.tile([128, NT * 128], f32, tag="dscr")
                # row norms: Act Square + DVE per-tile reduce
                nc.scalar.activation(sq[:], fb[:], AF.Square)
                nc.vector.tensor_reduce(
                    statv[:, 0:8],
                    sq[:].rearrange("p (t d) -> p t d", t=NT),
                    axis=mybir.AxisListType.X,
                    op=OP.add,
                )
                sroot = smpool.tile([128, 8], f32, tag="sroot")
                nc.scalar.activation(sroot[:], statv[:, 0:8], AF.Sqrt)
                rinv = smpool.tile([128, 8], f32, tag="rinv")
                nc.vector.reciprocal(rinv[:], sroot[:])

                # fn = f * rinv (bf16), per tile on Pool with per-row scale
                fn = fnpool.tile([128, NT * 128], bf16, tag="fn")
                for t in range(NT):
                    ts = slice(t * 128, (t + 1) * 128)
                    nc.gpsimd.tensor_scalar_mul(fn[:, ts], fb[:, ts], rinv[:, t:t + 1])

                # f16 cast (Pool), then fp8 hi/lo split of f16*w
                # (w = 0.5 for Act/Sign tiles 0..NA-1, 1.0 for DVE tiles)
                f16 = fnpool.tile([128, NT * 128], bf16, tag="f16")
                nc.gpsimd.tensor_copy(f16[:], fb[:])
                q8 = q8pool.tile([128, NT * 128], fp8, tag="q8")
                r8 = r8pool.tile([128, NT * 128], fp8, tag="r8")
                hi = slice(0, NA * 128)
                lo = slice(NA * 128, NT * 128)
                nc.gpsimd.tensor_scalar_mul(q8[:, hi], f16[:, hi], 0.5)
                nc.gpsimd.tensor_copy(q8[:, lo], f16[:, lo])
                nc.vector.scalar_tensor_tensor(
                    r8[:, hi], f16[:, hi], 0.5, q8[:, hi],
                    op0=OP.mult, op1=OP.subtract,
                )
                nc.vector.scalar_tensor_tensor(
                    r8[:, lo], f16[:, lo], 1.0, q8[:, lo],
                    op0=OP.mult, op1=OP.subtract,
                )

                # fnT / fT via DMA XBAR block transposes
                fnT = ftpool.tile([128, P], bf16, tag="fnT")
                fT = fTpool.tile([128, P], bf16, tag="fT")
                for t in range(NT):
                    ts = slice(t * 128, (t + 1) * 128)
                    nc.sync.dma_start_transpose(fnT[:, ts], fn[:, ts])
                    nc.sync.dma_start_transpose(fT[:, ts], f16[:, ts])
                v.update(statv=statv, q8=q8, r8=r8, fnT=fnT, fT=fT)

            def stage_sim(s):
                v = st[s]
                fnT, fT = v["fnT"], v["fT"]
                mask_t = mpool.tile([128, NT * P], fp8, tag="mask")
                dacc = smpool.tile([128, 8], f32, tag="dacc")
                for mt in MT_ORDER:
                    pss = pss_pool.tile([128, 1024], f32, tag="pss")
                    for nb in range(2):
                        nc.tensor.matmul(
                            pss[:, nb * 512:(nb + 1) * 512],
                            fnT[:, mt * 128:(mt + 1) * 128],
                            fnT[:, nb * 512:(nb + 1) * 512],
                            start=True,
                            stop=True,
                        )
                    blk = mask_t[:, mt * P:(mt + 1) * P]
                    if mt < NA:
                        nc.scalar.activation(
                            blk, pss[:], AF.Sign, bias=ntau[:],
                            accum_out=dacc[:, mt:mt + 1],
                        )
                    else:
                        nc.vector.tensor_scalar(
                            blk, pss[:], tau_bc[:], None,
                            op0=OP.is_gt, op1=OP.add,
                            accum_out=dacc[:, mt:mt + 1],
                        )

                # c_col = 0.5 * colsum of f16 over Act tiles (correction term)
                c_col = smpool.tile([128, 1], f32, tag="ccol")
                cscr = gscr.tile([128, NA * 128], bf16, tag="cscr")
                nc.scalar.activation(
                    cscr[:], fT[:, 0:NA * 128], AF.Copy, scale=0.5,
                    accum_out=c_col[:],
                )

                # deg: Act tiles hold sum(S) -> deg = 0.5*acc + 512; DVE exact
                deg = smpool.tile([128, 8], f32, tag="deg")
                nc.vector.tensor_scalar(
                    deg[:, 0:NA], dacc[:, 0:NA], 0.5, 512.0,
                    op0=OP.mult, op1=OP.add,
                )
                nc.vector.tensor_copy(deg[:, NA:8], dacc[:, NA:8])
                rdeg = smpool.tile([128, 8], f32, tag="rdeg")
                nc.vector.reciprocal(rdeg[:], deg[:])
                rrep = row_replicate(rdeg, s, "r")
                v.update(mask=mask_t, rrep=rrep, c_col=c_col)

            def stage_proto(s):
                v = st[s]
                statv, fT, c_col = v["statv"], v["fT"], v["c_col"]
                q8, r8, mask_t, rrep = v["q8"], v["r8"], v["mask"], v["rrep"]

                q8p = q8[:].rearrange("p (k d) -> p k d", k=NT)
                r8p = r8[:].rearrange("p (k d) -> p k d", k=NT)
                mkp = mask_t[:].rearrange("p (k q) -> p k q", k=NT)

                pT = pT_pool.tile([128, P], f32, tag="pT")
                for nb in range(2):
                    outp = pT[:, nb * 512:(nb + 1) * 512]
                    for j in range(4):
                        nc.tensor.matmul(
                            outp,
                            q8p[:, 2 * j:2 * j + 2, :],
                            mkp[:, 2 * j:2 * j + 2, nb * 512:(nb + 1) * 512],
                            perf_mode=DR,
                            start=(j == 0),
                            stop=False,
                        )
                    for j in range(4):
                        nc.tensor.matmul(
                            outp,
                            r8p[:, 2 * j:2 * j + 2, :],
                            mkp[:, 2 * j:2 * j + 2, nb * 512:(nb + 1) * 512],
                            perf_mode=DR,
                            start=False,
                            stop=(j == 3),
                        )

                # protoS = (protoT + c_col) * rdeg_rep (bf16), accum -> gtsum
                protoS = pspool.tile([128, P], bf16, tag="protoS")
                nc.vector.scalar_tensor_tensor(
                    protoS[:], pT[:], c_col[:], rrep[:],
                    op0=OP.add, op1=OP.mult,
                    accum_out=statv[:, 10:11],
                )
                # pf = sum protoS * fT  (per-d col)
                g1 = gscr.tile([128, P], bf16, tag="gscr")
                nc.vector.scalar_tensor_tensor(
                    g1[:], protoS[:], 1.0, fT[:],
                    op0=OP.mult, op1=OP.mult,
                    accum_out=statv[:, 8:9],
                )
                # pp = sum protoS^2  (per-d col)
                g2 = gscr.tile([128, P], bf16, tag="gscr")
                nc.scalar.activation(
                    g2[:], protoS[:], AF.Square, accum_out=statv[:, 9:10]
                )
                nc.scalar.copy(statall[:, s * SW:(s + 1) * SW], statv[:])
                del st[s]

            # software pipeline: load s+3 | prep s+2 | proto s | sim s+1
            # (proto emitted before sim so the PE has DoubleRow work queued
            # while the previous sim tiles' thresholds drain on Act/DVE)
            for k in range(BLOC + 3):
                if k < BLOC:
                    stage_load(k)
                if 1 <= k <= BLOC:
                    stage_prep(k - 1)
                if k >= 3:
                    stage_proto(k - 3)
                if 2 <= k <= BLOC + 1:
                    stage_sim(k - 2)

            nc.sync.dma_start(out_d[:], statall[:])

    nc.compile()
    return nc


def _get_program():
    global _PROG
    if _PROG is None:
        _PROG = _build_program()
    return _PROG


def _host_reduce(stats: np.ndarray) -> np.float32:
    """stats: [B, 128, SW] per-sample device stats -> scalar loss."""
    stats = stats.astype(np.float64)
    N = float(P * D)
    ff = stats[:, :, 0:8].sum(axis=(1, 2))       # Sum f^2
    pf = stats[:, :, 8].sum(axis=1)              # Sum protoS*fT
    pp = stats[:, :, 9].sum(axis=1)              # Sum protoS^2
    gtsum = stats[:, :, 10]                      # [B, D] Sum_q protoS

    mse = (pp - 2.0 * pf + ff) / N
    sum_proto = gtsum.sum(axis=1)
    gtm = gtsum / float(P)
    ybar = sum_proto / N
    S = ((gtm - ybar[:, None]) ** 2).sum(axis=1)
    sum_xc2 = pp - (sum_proto ** 2) / N
    num = float(P) * S
    corr = num / np.sqrt(sum_xc2 * num)
    loss = mse.mean() + (0.5 * (corr + 1.0)).mean()
    return np.float32(loss)


_LAST_RESULTS = None


def kernel(features: np.ndarray, tau: np.ndarray, **run_kwargs) -> np.ndarray:
    global _LAST_RESULTS
    from concourse import bass_utils

    features = np.ascontiguousarray(features, dtype=np.float32)
    tau_v = np.array(tau, dtype=np.float32).reshape(1, 1)

    nc = _get_program()
    shards = features.reshape(NCORES, BLOC, P, D)
    in_maps = [
        {"features": shards[i], "tau": tau_v.copy()} for i in range(NCORES)
    ]
    res = bass_utils.run_bass_kernel_spmd(
        nc, in_maps, core_ids=list(range(NCORES)), **run_kwargs
    )
    _LAST_RESULTS = res
    stats = np.concatenate(
        [
            res.results[i]["out"].reshape(128, BLOC, SW).transpose(1, 0, 2)
            for i in range(NCORES)
        ],
        axis=0,
    )
    return _host_reduce(stats)


if __name__ == "__main__":
    x = np.random.randn(B, P, D).astype(np.float32)
    t = np.float32(0.5)
    print(kernel(x, t))
